# revision 32
# baseline (speedup 1.0000x reference)
"""Trainium2 Bass kernel for nn_AttentionSeqModel (GRU encoder + attention GRU decoder).

Strategy: data-parallel over batch across 8 cores (64 rows/core + global batch
row 0 appended as column 64 so each core computes enc_outs locally).
Column-major activation layout [feature -> partitions, batch -> free dim].

v2 optimizations over baseline:
- Decoder uses ONLY exp/ln/identity activations (GRU sigmoid/tanh rewritten in
  terms of exp + DVE/Pool rational ops) so the Activation engine never switches
  its function table (1283ns per switch, 4 switches/step in the baseline).
- Merged activations (sigmoid over r&z gates in one op; exp over r&z in one op).
- Gate biases accumulated in PSUM via tiny K=1/K=2 matmuls instead of extra
  activation ops.
- Decoder feeds back pre-log-softmax logits y plus the log-partition srow
  separately (corrections folded into score/comb matmuls) to shorten the
  serial chain through exp/sum/ln.
- Encoder x-projections software-pipelined one step ahead of the h-recurrence.
- Independent batch-column chains per core (encoder 2, decoder 4) to hide
  per-op latency; PSUM hand-packed into the 8 banks (region-granular deps).
- Element-wise work spread across DVE and GpSimd engines.
"""

import numpy as np

B, L, D, H, A = 512, 512, 128, 128, 16
NCORES = 8
BS = B // NCORES  # 64 batch rows per core
BC = BS + 1       # + batch row 0 (for enc_outs replication)
CH = 8            # obs timesteps per DMA chunk
NCH = L // H      # 4 attention chunks

ENC_CHAINS = 2
DEC_CHAINS = 2

USE_BF16 = True

_CACHE = {}


def _build_program():
    import concourse.bass as bass
    import concourse.bacc as bacc
    import concourse.tile as tile
    import concourse.mybir as mybir

    f32 = mybir.dt.float32
    wdt = mybir.dt.bfloat16 if USE_BF16 else f32
    AF = mybir.ActivationFunctionType
    OP = mybir.AluOpType

    nc = bacc.Bacc()

    def dp(name, shape, dt):
        return nc.declare_dram_parameter(name, list(shape), dt, isOutput=False)

    obs_d = dp("obs", [L, D, BC], wdt)
    encfW_d = dp("enc_f_WT", [D, 3 * H], wdt)
    encWhh_d = dp("enc_WhhT", [H, 3 * H], wdt)
    attnf1_d = dp("attn_f1T", [A, L], wdt)
    attnW2_d = dp("attn_W2T", [H, L], wdt)
    attnk2_d = dp("attn_k2", [2, L], wdt)       # rows: -c1, attn bias
    combf1_d = dp("comb_f1T", [A, H], wdt)
    combW2_d = dp("comb_W2T", [H, H], wdt)
    combk2_d = dp("comb_k2", [2, H], wdt)       # rows: -c2, 0
    decWih_d = dp("dec_WihT", [H, 3 * H], wdt)
    decWhh_d = dp("dec_WhhT", [H, 3 * H], wdt)
    outW_d = dp("out_WT", [H, A], wdt)
    encbr_d = dp("enc_br", [1, H], f32)
    encbz_d = dp("enc_bz", [1, H], f32)
    encb_d = dp("enc_bias", [H, 2], f32)        # cols: inn, hn bias
    decbr_d = dp("dec_br", [1, H], f32)
    decbz_d = dp("dec_bz", [1, H], f32)
    decb_d = dp("dec_bias", [H, 3], f32)        # cols: inn, hn, comb bias
    outb_d = dp("out_bias", [A, 1], f32)
    ident_d = dp("ident", [H, H], wdt)
    out_d = nc.declare_dram_parameter("out", [BS, A], f32, isOutput=True)

    # encoder chain column ranges (chain 1 owns global col 64 = batch row 0)
    ENC_COLS = [(0, 32), (32, BC)]
    DW = BS // DEC_CHAINS
    DEC_COLS = [(i * DW, (i + 1) * DW) for i in range(DEC_CHAINS)]

    with tile.TileContext(nc) as tc:
        with (
            tc.tile_pool(name="const", bufs=1) as constp,
            tc.tile_pool(name="obsp", bufs=3) as obsp,
            tc.tile_pool(name="state", bufs=2) as statep,
            tc.tile_pool(name="work", bufs=2) as workp,
        ):
            # ---- load constants ----
            def cload(dram, shape, dt, tag):
                t = constp.tile(shape, dt, tag=tag)
                nc.sync.dma_start(out=t, in_=dram[:])
                return t

            encfW_s = cload(encfW_d, [D, 3 * H], wdt, "encfW")
            encWhh_s = cload(encWhh_d, [H, 3 * H], wdt, "encWhh")
            attnf1_s = cload(attnf1_d, [A, L], wdt, "attnf1")
            attnW2_s = cload(attnW2_d, [H, L], wdt, "attnW2")
            attnk2_s = cload(attnk2_d, [2, L], wdt, "attnk2")
            combf1_s = cload(combf1_d, [A, H], wdt, "combf1")
            combW2_s = cload(combW2_d, [H, H], wdt, "combW2")
            combk2_s = cload(combk2_d, [2, H], wdt, "combk2")
            decWih_s = cload(decWih_d, [H, 3 * H], wdt, "decWih")
            decWhh_s = cload(decWhh_d, [H, 3 * H], wdt, "decWhh")
            outW_s = cload(outW_d, [H, A], wdt, "outW")
            encbr_s = cload(encbr_d, [1, H], f32, "encbr")
            encbz_s = cload(encbz_d, [1, H], f32, "encbz")
            encb_s = cload(encb_d, [H, 2], f32, "encb")
            decbr_s = cload(decbr_d, [1, H], f32, "decbr")
            decbz_s = cload(decbz_d, [1, H], f32, "decbz")
            decb_s = cload(decb_d, [H, 3], f32, "decb")
            outb_s = cload(outb_d, [A, 1], f32, "outb")
            ident_s = cload(ident_d, [H, H], wdt, "ident")

            ones_s = constp.tile([H, H], wdt)
            nc.vector.memset(ones_s, 1.0)
            ones16_s = constp.tile([A, A], wdt)
            nc.vector.memset(ones16_s, 1.0)
            onesrow_f = constp.tile([1, BC], f32)
            nc.vector.memset(onesrow_f, 1.0)

            enc_outs_cm = constp.tile([H, L], wdt)
            enc_outs_rm = constp.tile([H, L], wdt)

            # =========================== ENCODER ===========================
            enc_h = []
            for c, (c0, c1) in enumerate(ENC_COLS):
                w = c1 - c0
                h = statep.tile([H, w], wdt, tag=f"eh{c}")
                nc.vector.memset(h, 0.0)
                enc_h.append(h)

            with tc.tile_pool(name="encps", bufs=2, space="PSUM") as encps:
                x_chunks = {}

                def dma_chunk(ci):
                    xt = obsp.tile([D, CH, BC], wdt, tag="x")
                    nc.sync.dma_start(
                        out=xt,
                        in_=obs_d[ci * CH:(ci + 1) * CH].rearrange(
                            "t d b -> d t b"))
                    x_chunks[ci] = xt

                pend = {}

                def emit_x(c, t):
                    """x-side matmuls (+ r/z bias) for chain c, step t.
                    Bank layout (f32 cols): [gr | gz | inn | hn] each w wide."""
                    if t >= L:
                        return
                    c0, c1 = ENC_COLS[c]
                    w = c1 - c0
                    ci, j = divmod(t, CH)
                    x = x_chunks[ci][:, j, c0:c1]
                    bk = encps.tile([H, 4 * w], f32, tag=f"egb{c}")
                    nc.tensor.matmul(bk[:, 0:w], encfW_s[:, 0:H], x,
                                     start=True, stop=False)
                    nc.tensor.matmul(bk[:, w:2 * w], encfW_s[:, H:2 * H], x,
                                     start=True, stop=False)
                    nc.tensor.matmul(bk[:, 0:w], encbr_s, onesrow_f[:, c0:c1],
                                     start=False, stop=False)
                    nc.tensor.matmul(bk[:, w:2 * w], encbz_s,
                                     onesrow_f[:, c0:c1],
                                     start=False, stop=False)
                    nc.tensor.matmul(bk[:, 2 * w:3 * w],
                                     encfW_s[:, 2 * H:3 * H], x,
                                     start=True, stop=True)
                    pend[(c, t)] = bk

                dma_chunk(0)
                for c in range(ENC_CHAINS):
                    emit_x(c, 0)

                est = [dict() for _ in range(ENC_CHAINS)]

                def enc_e1(c, t):
                    c0, c1 = ENC_COLS[c]
                    w = c1 - c0
                    h = enc_h[c]
                    bk = pend.pop((c, t))
                    nc.tensor.matmul(bk[:, 0:w], encWhh_s[:, 0:H], h,
                                     start=False, stop=True)
                    nc.tensor.matmul(bk[:, w:2 * w],
                                     encWhh_s[:, H:2 * H], h,
                                     start=False, stop=True)
                    nc.tensor.matmul(bk[:, 3 * w:4 * w],
                                     encWhh_s[:, 2 * H:3 * H], h,
                                     start=True, stop=True)
                    rz = workp.tile([H, 2 * w], f32, tag=f"erz{c}")
                    nc.scalar.activation(rz, bk[:, 0:2 * w], AF.Sigmoid)
                    tmp = workp.tile([H, w], f32, tag=f"etmp{c}")
                    nc.vector.scalar_tensor_tensor(
                        tmp, bk[:, 3 * w:4 * w], encb_s[:, 1:2],
                        rz[:, 0:w], OP.add, OP.mult)
                    pre = workp.tile([H, w], f32, tag=f"epre{c}")
                    nc.vector.scalar_tensor_tensor(
                        pre, bk[:, 2 * w:3 * w], encb_s[:, 0:1],
                        tmp, OP.add, OP.add)
                    est[c] = dict(rz=rz, pre=pre, h=h)

                def enc_e2(c, t):
                    c0, c1 = ENC_COLS[c]
                    w = c1 - c0
                    rz = est[c]["rz"]
                    pre = est[c]["pre"]
                    h = est[c]["h"]
                    n = workp.tile([H, w], f32, tag=f"en{c}")
                    nc.scalar.activation(n, pre, AF.Tanh)
                    zh = workp.tile([H, w], f32, tag=f"ezh{c}")
                    nc.gpsimd.tensor_tensor(zh, rz[:, w:2 * w], h, OP.mult)
                    u = workp.tile([H, w], f32, tag=f"eu{c}")
                    nc.gpsimd.tensor_scalar(u, rz[:, w:2 * w],
                                            -1.0, 1.0, OP.mult, OP.add)
                    v = workp.tile([H, w], f32, tag=f"ev{c}")
                    nc.vector.tensor_tensor(v, n, u, OP.mult)
                    h_new = statep.tile([H, w], wdt, tag=f"eh{c}")
                    nc.vector.tensor_tensor(h_new, v, zh, OP.add)
                    enc_h[c] = h_new
                    if c == ENC_CHAINS - 1:
                        nc.gpsimd.tensor_copy(
                            enc_outs_cm[:, t:t + 1], h_new[:, w - 1:w])

                for ci in range(L // CH):
                    if ci + 1 < L // CH:
                        dma_chunk(ci + 1)
                    for j in range(CH):
                        t = ci * CH + j
                        for c in range(ENC_CHAINS):
                            emit_x(c, t + 1)
                        enc_e1(0, t)
                        if t > 0:
                            enc_e2(1, t - 1)
                        enc_e2(0, t)
                        enc_e1(1, t)
                enc_e2(1, L - 1)

                # ---- transpose enc_outs (column-major -> row-major) ----
                for ch in range(NCH):
                    cs = slice(ch * H, (ch + 1) * H)
                    tp = encps.tile([H, H], wdt, tag="tp")
                    nc.tensor.transpose(tp, enc_outs_cm[:, cs], ident_s)
                    nc.scalar.activation(enc_outs_rm[:, cs], tp, AF.Copy)

            # =========================== DECODER ===========================
            dec_h = []
            dec_y = []
            dec_rhs2 = []
            for c, (c0, c1) in enumerate(DEC_COLS):
                w = c1 - c0
                h = statep.tile([H, w], wdt, tag=f"dh{c}")
                src = []
                pos = c0
                while pos < c1:
                    for ec, (e0, e1) in enumerate(ENC_COLS):
                        if e0 <= pos < e1:
                            take = min(c1, e1) - pos
                            src.append((ec, pos - e0, pos - c0, take))
                            pos += take
                            break
                for ec, s0, d0, take in src:
                    nc.scalar.activation(h[:, d0:d0 + take],
                                         enc_h[ec][:, s0:s0 + take], AF.Copy)
                y = statep.tile([A, w], wdt, tag=f"dy{c}")
                nc.vector.memset(y, 0.0)
                # r2 rows: [srow; ones] (srow at partition 0 for Ln writes)
                r2 = constp.tile([2, w], wdt, tag=f"r2{c}")
                nc.vector.memset(r2, 1.0)
                nc.vector.memset(r2[0:1, :], 0.0)
                dec_h.append(h)
                dec_y.append(y)
                dec_rhs2.append(r2)

            with tc.tile_pool(name="decps", bufs=2, space="PSUM") as decps:
                # Half-step software pipeline across the two chains:
                #   c0.H1(t), c1.H2(t-1), c0.H2(t), c1.H1(t)
                # H1 = attention+comb+relu, H2 = GRU tail + logits/srow.
                st = [dict() for _ in range(DEC_CHAINS)]

                def emit_h1(c, t):
                    c0, c1 = DEC_COLS[c]
                    w = c1 - c0
                    h = dec_h[c]
                    y_bf = dec_y[c]
                    r2 = dec_rhs2[c]
                    # bank A cols: [ s(4w) | gr(w) | gz(w) | inn(w) | hn(w) ]
                    bA = decps.tile([H, 8 * w], f32, tag=f"bA{c}")
                    # bank B cols: [ sum(w) | app(w) | o(w) | y(w) | ls(w) ]
                    bB = decps.tile([H, 5 * w], f32, tag=f"bB{c}")
                    s_ps = bA[:, 0:4 * w]
                    sum_ps = bB[:, 0:w]
                    app_ps = bB[:, w:2 * w]
                    o_ps = bB[:, 2 * w:3 * w]
                    for ch in range(NCH):
                        cs = slice(ch * H, (ch + 1) * H)
                        reg = s_ps[:, ch * w:(ch + 1) * w]
                        nc.tensor.matmul(reg, attnf1_s[:, cs], y_bf,
                                         start=True, stop=False)
                        nc.tensor.matmul(reg, attnW2_s[:, cs], h,
                                         start=False, stop=False)
                        nc.tensor.matmul(reg, attnk2_s[:, cs], r2,
                                         start=False, stop=True)
                    aw = workp.tile([H, NCH * w], wdt, tag=f"aw{c}")
                    nc.scalar.activation(aw, s_ps, AF.Exp)
                    for ch in range(NCH):
                        cs = slice(ch * H, (ch + 1) * H)
                        reg = aw[:, ch * w:(ch + 1) * w]
                        nc.tensor.matmul(sum_ps, ones_s, reg,
                                         start=(ch == 0), stop=(ch == NCH - 1))
                        nc.tensor.matmul(app_ps, enc_outs_rm[:, cs], reg,
                                         start=(ch == 0), stop=(ch == NCH - 1))
                    rec = workp.tile([H, w], f32, tag=f"rec{c}")
                    nc.vector.reciprocal(rec, sum_ps)
                    an = workp.tile([H, w], wdt, tag=f"an{c}")
                    nc.vector.tensor_tensor(an, app_ps, rec, OP.mult)
                    nc.tensor.matmul(o_ps, combf1_s, y_bf,
                                     start=True, stop=False)
                    nc.tensor.matmul(o_ps, combW2_s, an,
                                     start=False, stop=False)
                    nc.tensor.matmul(o_ps, combk2_s, r2,
                                     start=False, stop=True)
                    o_bf = workp.tile([H, w], wdt, tag=f"o{c}")
                    nc.scalar.activation(o_bf, o_ps, AF.Relu,
                                         bias=decb_s[:, 2:3])
                    st[c] = dict(bA=bA, bB=bB, o_bf=o_bf, h=h)

                def emit_h2(c, t):
                    c0, c1 = DEC_COLS[c]
                    w = c1 - c0
                    r2 = dec_rhs2[c]
                    bA = st[c]["bA"]
                    bB = st[c]["bB"]
                    o_bf = st[c]["o_bf"]
                    h = st[c]["h"]
                    y_ps = bB[0:A, 3 * w:4 * w]
                    ls_ps = bB[0:A, 4 * w:5 * w]
                    gr = bA[:, 4 * w:5 * w]
                    gz = bA[:, 5 * w:6 * w]
                    grz = bA[:, 4 * w:6 * w]
                    inn_ps = bA[:, 6 * w:7 * w]
                    hn_ps = bA[:, 7 * w:8 * w]
                    nc.tensor.matmul(gr, decWih_s[:, 0:H], o_bf,
                                     start=True, stop=False)
                    nc.tensor.matmul(gz, decWih_s[:, H:2 * H], o_bf,
                                     start=True, stop=False)
                    nc.tensor.matmul(gr, decbr_s, onesrow_f[:, c0:c1],
                                     start=False, stop=False)
                    nc.tensor.matmul(gz, decbz_s, onesrow_f[:, c0:c1],
                                     start=False, stop=False)
                    nc.tensor.matmul(gr, decWhh_s[:, 0:H], h,
                                     start=False, stop=True)
                    nc.tensor.matmul(gz, decWhh_s[:, H:2 * H], h,
                                     start=False, stop=True)
                    nc.tensor.matmul(inn_ps, decWih_s[:, 2 * H:3 * H], o_bf,
                                     start=True, stop=True)
                    nc.tensor.matmul(hn_ps, decWhh_s[:, 2 * H:3 * H], h,
                                     start=True, stop=True)
                    Erz = workp.tile([H, 2 * w], f32, tag=f"Erz{c}")
                    nc.scalar.activation(Erz, grz, AF.Exp, scale=-1.0)
                    trz = workp.tile([H, 2 * w], f32, tag=f"trz{c}")
                    nc.vector.tensor_scalar_add(trz, Erz, 1.0)
                    rt = workp.tile([H, 2 * w], f32, tag=f"rt{c}")
                    nc.vector.reciprocal(rt, trz)
                    tmp = workp.tile([H, w], f32, tag=f"dtmp{c}")
                    nc.vector.scalar_tensor_tensor(
                        tmp, hn_ps, decb_s[:, 1:2], rt[:, 0:w],
                        OP.add, OP.mult)
                    pre = workp.tile([H, w], f32, tag=f"dpre{c}")
                    nc.vector.scalar_tensor_tensor(
                        pre, inn_ps, decb_s[:, 0:1], tmp, OP.add, OP.add)
                    # n = tanh(pre) via Pade (|pre| < 1 here)
                    x2p = workp.tile([H, w], f32, tag=f"x2p{c}")
                    nc.vector.tensor_tensor(x2p, pre, pre, OP.mult)
                    tn1 = workp.tile([H, w], f32, tag=f"tn1{c}")
                    nc.vector.scalar_tensor_tensor(
                        tn1, x2p, 27.0, pre, OP.add, OP.mult)
                    tden = workp.tile([H, w], f32, tag=f"tden{c}")
                    nc.vector.tensor_scalar(tden, x2p, 9.0, 27.0,
                                            OP.mult, OP.add)
                    rden = workp.tile([H, w], f32, tag=f"rden{c}")
                    nc.vector.reciprocal(rden, tden)
                    n = workp.tile([H, w], f32, tag=f"n{c}")
                    nc.vector.tensor_tensor(n, tn1, rden, OP.mult)
                    g2 = workp.tile([H, w], f32, tag=f"g2{c}")
                    nc.gpsimd.tensor_tensor(g2, Erz[:, w:2 * w],
                                            rt[:, w:2 * w], OP.mult)
                    hh = workp.tile([H, w], wdt, tag=f"hh{c}")
                    nc.gpsimd.tensor_tensor(hh, h, rt[:, w:2 * w], OP.mult)
                    v = workp.tile([H, w], wdt, tag=f"v{c}")
                    nc.vector.tensor_tensor(v, n, g2, OP.mult)
                    h_new = statep.tile([H, w], wdt, tag=f"dh{c}")
                    nc.vector.tensor_tensor(h_new, v, hh, OP.add)
                    dec_h[c] = h_new
                    # y = outW*(v+hh): start the srow path from v, not h'
                    nc.tensor.matmul(y_ps, outW_s, v, start=True, stop=False)
                    nc.tensor.matmul(y_ps, outW_s, hh, start=False, stop=True)
                    elg = workp.tile([A, w], wdt, tag=f"elg{c}")
                    nc.scalar.activation(elg, y_ps, AF.Exp,
                                         bias=outb_s[:, 0:1])
                    nc.tensor.matmul(ls_ps, ones16_s, elg,
                                     start=True, stop=True)
                    if t < L - 1:
                        nc.scalar.activation(r2[0:1, :], ls_ps[0:1, :], AF.Ln)
                        y_new = statep.tile([A, w], wdt, tag=f"dy{c}")
                        nc.vector.tensor_scalar_add(y_new, y_ps,
                                                    outb_s[:, 0:1])
                        dec_y[c] = y_new
                    else:
                        lls = workp.tile([A, w], f32, tag=f"lls{c}")
                        nc.scalar.activation(lls, ls_ps, AF.Ln)
                        lgf = workp.tile([A, w], f32, tag=f"lgf{c}")
                        nc.vector.scalar_tensor_tensor(
                            lgf, y_ps, outb_s[:, 0:1], lls,
                            OP.add, OP.subtract)
                        nc.sync.dma_start(
                            out=out_d[c0:c1].rearrange("b a -> a b"),
                            in_=lgf)

                for t in range(L):
                    emit_h1(0, t)
                    if t > 0:
                        emit_h2(1, t - 1)
                    emit_h2(0, t)
                    emit_h1(1, t)
                emit_h2(1, L - 1)
    nc.compile()
    _dedupe_act_table_loads(nc)
    return nc


def _dedupe_act_table_loads(nc):
    """The framework's table-choice pass greedily loads the FIRST table
    containing each activation function, so a decoder alternating Exp and Ln
    ping-pongs between `exp_and_others` (0) and `natural_log` (5) at 1283ns
    per load. Both functions (plus copy/identity/relu used here) live in
    `natural_log_exp_and_others`, so rewrite the first such load to that
    table and drop the rest."""
    import concourse.mybir as mybir
    from concourse.hw_specs import get_activation_tables

    tables = list(get_activation_tables(nc.m.arch).items())
    by_name = {name: i for i, (name, _) in enumerate(tables)}
    combined = by_name["natural_log_exp_and_others"]
    comb_set = tables[combined][1]
    redundant = {by_name["exp_and_others"], by_name["natural_log"]}
    for blk in nc.m.functions[0].blocks:
        first = True
        new_insts = []
        for inst in blk.instructions:
            if isinstance(inst, mybir.InstLoadActFuncSet) and \
                    inst.act_func_set_id in redundant:
                if first:
                    inst.act_func_set_id = combined
                    first = False
                    new_insts.append(inst)
                # else: drop redundant load
            else:
                new_insts.append(inst)
        blk.instructions[:] = new_insts
    # validate: every activation must be served by the table loaded before it
    for blk in nc.m.functions[0].blocks:
        cur = None
        for inst in blk.instructions:
            if isinstance(inst, mybir.InstLoadActFuncSet):
                cur = tables[inst.act_func_set_id][1]
            elif isinstance(inst, mybir.InstActivation):
                fname = str(inst.func).split(".")[-1].lower()
                if cur is not None:
                    served = any(fname == str(f).split(".")[-1].lower()
                                 for f in cur)
                    assert served, (fname, inst.name)


def _prep_inputs(inputs):
    import ml_dtypes
    bf16 = ml_dtypes.bfloat16
    wnp = bf16 if USE_BF16 else np.float32

    f = {k: np.asarray(v, dtype=np.float32) for k, v in inputs.items()}
    obs = f["obs"]

    enc_f_W = f["enc_Wih"] @ f["enc_emb_W"]                 # (3H, D)
    enc_b_f = f["enc_Wih"] @ f["enc_emb_b"] + f["enc_bih"]  # (3H,)
    enc_br = (enc_b_f[0:H] + f["enc_bhh"][0:H])[None, :].astype(np.float32)
    enc_bz = (enc_b_f[H:2 * H] + f["enc_bhh"][H:2 * H])[None, :].astype(np.float32)
    enc_bias = np.stack([
        enc_b_f[2 * H:3 * H],
        f["enc_bhh"][2 * H:3 * H],
    ], axis=1).astype(np.float32)                           # (H, 2)

    attn_f1 = f["attn_W"][:, :H] @ f["dec_emb_W"]           # (L, A)
    attn_bias = (f["attn_W"][:, :H] @ f["dec_emb_b"] + f["attn_b"])  # (L,)
    c1 = attn_f1.sum(axis=1)                                # (L,)
    attn_k2 = np.stack([-c1, attn_bias], axis=0).astype(wnp)  # (2, L)
    comb_f1 = f["comb_W"][:, :H] @ f["dec_emb_W"]           # (H, A)
    comb_bf = f["comb_W"][:, :H] @ f["dec_emb_b"] + f["comb_b"]      # (H,)
    c2 = comb_f1.sum(axis=1)                                # (H,)
    comb_k2 = np.stack([-c2, np.zeros(H)], axis=0).astype(wnp)  # (2, H)
    dec_br = (f["dec_bih"][0:H] + f["dec_bhh"][0:H])[None, :].astype(np.float32)
    dec_bz = (f["dec_bih"][H:2 * H] + f["dec_bhh"][H:2 * H])[None, :].astype(np.float32)
    dec_bias = np.stack([
        f["dec_bih"][2 * H:3 * H],
        f["dec_bhh"][2 * H:3 * H],
        comb_bf,
    ], axis=1).astype(np.float32)                           # (H, 3)

    shared = {
        "enc_f_WT": np.ascontiguousarray(enc_f_W.T, dtype=wnp),
        "enc_WhhT": np.ascontiguousarray(f["enc_Whh"].T, dtype=wnp),
        "attn_f1T": np.ascontiguousarray(attn_f1.T, dtype=wnp),
        "attn_W2T": np.ascontiguousarray(f["attn_W"][:, H:].T, dtype=wnp),
        "attn_k2": attn_k2,
        "comb_f1T": np.ascontiguousarray(comb_f1.T, dtype=wnp),
        "comb_W2T": np.ascontiguousarray(f["comb_W"][:, H:].T, dtype=wnp),
        "comb_k2": comb_k2,
        "dec_WihT": np.ascontiguousarray(f["dec_Wih"].T, dtype=wnp),
        "dec_WhhT": np.ascontiguousarray(f["dec_Whh"].T, dtype=wnp),
        "out_WT": np.ascontiguousarray(f["out_W"].T, dtype=wnp),
        "enc_br": enc_br,
        "enc_bz": enc_bz,
        "enc_bias": enc_bias,
        "dec_br": dec_br,
        "dec_bz": dec_bz,
        "dec_bias": dec_bias,
        "out_bias": np.ascontiguousarray(f["out_b"][:, None], dtype=np.float32),
        "ident": np.eye(H, dtype=wnp),
    }

    # one global transpose to (L, D, B) in bf16, then cheap per-core slices
    obs_t = np.ascontiguousarray(obs.astype(wnp).transpose(1, 2, 0))
    in_maps = []
    for c in range(NCORES):
        ob = np.concatenate(
            [obs_t[:, :, c * BS:(c + 1) * BS], obs_t[:, :, 0:1]], axis=2)
        m = dict(shared)
        m["obs"] = np.ascontiguousarray(ob)
        in_maps.append(m)
    return in_maps


def _get_program():
    if "nc" not in _CACHE:
        _CACHE["nc"] = _build_program()
    return _CACHE["nc"]


def kernel(_trace=False, **inputs):
    from concourse.bass_utils import run_bass_kernel_spmd

    nc = _get_program()
    in_maps = _prep_inputs(inputs)
    res = run_bass_kernel_spmd(nc, in_maps, list(range(NCORES)), trace=_trace)
    _CACHE["last_results"] = res
    out = np.concatenate([res.results[i]["out"] for i in range(NCORES)], axis=0)
    return out.astype(np.float32)
